# revision 12
# baseline (speedup 1.0000x reference)
"""3D Haar DWT (nn_Patcher) Trainium2 Bass kernel.

Math: with even dims and reflect-pad(0,1) never engaged, the reference is
non-overlapping 2x2x2 Haar butterflies; r^3 * 2*sqrt(2) == 1 exactly, so the
output is pure +/-1 sums over 2x2x2 blocks -- no multiplies needed.

Full input  x  [4, 3, 32, 256, 256] f32
Full output y  [4, 24, 16, 128, 128] f32   (8 subbands x 3 ch on channel dim)

Sharding (8 cores): core k -> (b = k//2, th = k%2); shard input
x[b, :, 16*th : 16*th+16]  -> [3, 16, 256, 256]  (12.58 MB)
shard output -> y[b, :, 8*th : 8*th+8]           -> [24, 8, 128, 128]

Per-core kernel: 6 mega-iters over (c in 3, tg in 2). Partition p = t*32 + h4
(t = output frame in mega-iter 0..3, h4 = h_out//4 0..31). All three Haar
stages stay within-partition:
  T-stage: lo on DVE, hi on Pool      (frame parity, contiguous FD=2048)
  H-stage: lo on DVE, hi on Pool      (row parity)
  W-stage: hi on Pool; lo alternates DVE/Pool per iter (engine balance)
DMA: each engine's HWDGE ring serializes its transfers, so input DMAs
alternate SP/Act and output DMAs alternate Act/SP to use both rings and
let outputs start as soon as each mega-iter finishes.
"""

import sys

for _p in ("/opt/trn_rl_repo", "/opt/pypackages"):
    if _p not in sys.path:
        sys.path.append(_p)

import numpy as np

_NC_CACHE = {}


def _build(reps=1):
    if reps in _NC_CACHE:
        return _NC_CACHE[reps]

    from concourse import bacc, mybir
    from concourse.tile import TileContext

    fp32 = mybir.dt.float32
    add = mybir.AluOpType.add
    sub = mybir.AluOpType.subtract

    # Bacc (not plain Bass): its finalize() runs the backend passes, incl.
    # generate_event_semaphores which splits multi-wait instructions (HW
    # allows at most 1 sync wait per instruction).
    nc = bacc.Bacc(None, target_bir_lowering=False)
    x = nc.dram_tensor("x_shard", [3, 16, 256, 256], fp32, kind="ExternalInput")
    y = nc.dram_tensor("y_shard", [24, 8, 128, 128], fp32, kind="ExternalOutput")

    # y viewed as [c, tg, (t h4), s, (hr w)] ; channels 24 = (s c), s=4tb+2hb+wb.
    # (t h4) merges to one stride-512 dim matching SBUF partitions; (hr w) is a
    # contiguous 2 KB run. 3-dim DMA AP both sides, partition-first on SBUF.
    yv = y[:].rearrange(
        "(s c) (tg t) (h4 hr) w -> c tg (t h4) s (hr w)", s=8, c=3, tg=2, hr=4
    )

    in_cycle = [nc.sync, nc.scalar]
    out_cycle = [nc.scalar, nc.sync]
    ni = 0
    no = 0

    with TileContext(nc) as tc:
        with tc.tile_pool(name="pool", bufs=2) as pool:
            it = 0
            for _rep in range(reps):
                for c in range(3):
                    for tg in range(2):
                        # tiles: [partition=128, ...free dims...], 2 MB each
                        t_in = pool.tile([128, 2, 4, 2, 256], fp32)  # (f, q, rp, w)
                        t_t = pool.tile([128, 2, 4, 2, 256], fp32)   # (tb, q, rp, w)
                        t_h = pool.tile([128, 2, 2, 4, 256], fp32)   # (tb, hb, q, w)
                        t_o = pool.tile([128, 2, 2, 2, 4, 128], fp32)  # (tb,hb,wb,q,w)

                        # ---- input DMA: 2 transfers of 1 MB (8 KB runs),
                        # split by frame parity so partition (t h4) merges ----
                        f0 = 8 * tg
                        for f in range(2):
                            src = x[c, f0 + f : f0 + 8 : 2].rearrange(
                                "t (h4 r) w -> t h4 (r w)", h4=32
                            )
                            dst = t_in[:, f].rearrange("p q r w -> p (q r w)")
                            in_cycle[ni % 2].dma_start(out=dst, in_=src)
                            ni += 1

                        V = nc.vector
                        P = nc.gpsimd

                        # ---- T stage (frame parity) ----
                        V.tensor_tensor(
                            out=t_t[:, 0], in0=t_in[:, 0], in1=t_in[:, 1], op=add
                        )
                        P.tensor_tensor(
                            out=t_t[:, 1], in0=t_in[:, 0], in1=t_in[:, 1], op=sub
                        )

                        # ---- H stage (row parity rp) ----
                        V.tensor_tensor(
                            out=t_h[:, :, 0],
                            in0=t_t[:, :, :, 0],
                            in1=t_t[:, :, :, 1],
                            op=add,
                        )
                        P.tensor_tensor(
                            out=t_h[:, :, 1],
                            in0=t_t[:, :, :, 0],
                            in1=t_t[:, :, :, 1],
                            op=sub,
                        )

                        # ---- W stage (column parity) ----
                        t_hv = t_h.rearrange(
                            "p a b q (wh wl) -> p a b q wh wl", wl=2
                        )
                        w0e = V if it % 2 == 0 else P
                        w0e.tensor_tensor(
                            out=t_o[:, :, :, 0],
                            in0=t_hv[:, :, :, :, :, 0],
                            in1=t_hv[:, :, :, :, :, 1],
                            op=add,
                        )
                        P.tensor_tensor(
                            out=t_o[:, :, :, 1],
                            in0=t_hv[:, :, :, :, :, 0],
                            in1=t_hv[:, :, :, :, :, 1],
                            op=sub,
                        )

                        # ---- output DMA: 1 transfer of 2 MB (2 KB runs) ----
                        src = t_o.rearrange("p a b v q w -> p (a b v) (q w)")
                        out_cycle[no % 2].dma_start(out=yv[c, tg], in_=src)
                        no += 1
                        it += 1

    nc.finalize()
    _NC_CACHE[reps] = nc
    return nc


def _run(x, trace=False, **spmd_kwargs):
    from concourse.bass_utils import run_bass_kernel_spmd

    x = np.ascontiguousarray(np.asarray(x, dtype=np.float32))
    assert x.shape == (4, 3, 32, 256, 256), x.shape

    nc = _build()
    in_maps = []
    for k in range(8):
        b, th = divmod(k, 2)
        in_maps.append(
            {"x_shard": np.ascontiguousarray(x[b, :, 16 * th : 16 * th + 16])}
        )

    bkr = run_bass_kernel_spmd(nc, in_maps, list(range(8)), trace=trace, **spmd_kwargs)

    out = np.empty((4, 24, 16, 128, 128), dtype=np.float32)
    for k in range(8):
        b, th = divmod(k, 2)
        out[b, :, 8 * th : 8 * th + 8] = np.asarray(bkr.results[k]["y_shard"])
    return out, bkr


def kernel(x):
    out, _ = _run(x)
    return out
